# revision 5
# baseline (speedup 1.0000x reference)
"""BertSelfAttention (group_prob-scaled probs, mask|diag masking) on 8 TRN2 cores.

Sharding: data-parallel over batch (16 -> 2 per core). Device math per (b, head):
  qT/kT = W^T-layout projections of hs^T (f32r matmuls, full PE rate)
  scores = qT_h^T @ kT_h   (1/8 folded into Wq on host)
  masked = scores + M      (M in {0, -inf}; exact -inf via DVE add)
  p = exp(masked) [bf16] with per-row accumulated sum (ACT)
  pgT = transpose(p) * gpT [bf16]  (PE transpose + fused DVE multiply)
  ctxT_h = v_h^T @ pg^T  as [d, i] (bf16 matmul, N=512, fp32 accumulate)
  ctx = transpose(ctxT pair) / rowsum  (PE transpose + ACT scale-copy)
Host side is layout-only: transposes (hs^T, W^T, gp^T), 1/8 scale fold (exact,
power of two), additive mask build, bf16 casts for the probability path.
"""

import os
import sys

import numpy as np

for _p in ("/opt/trn_rl_repo", "/root/.axon_site/_ro/trn_rl_repo"):
    if _p not in sys.path and os.path.isdir(_p):
        sys.path.append(_p)

import ml_dtypes
import concourse.bacc as bacc
import concourse.bass as bass
import concourse.tile as tile
from concourse import mybir
from concourse.bass_utils import run_bass_kernel_spmd
from concourse.masks import make_identity

NB = 2          # batches per core
S = 512         # sequence length
H = 1024        # hidden
NH = 16         # heads
DH = 64         # head dim
NCORES = 8
HC = H // 128   # 8 hidden chunks
SB = S // 128   # 4 seq blocks

F32 = mybir.dt.float32
F32R = mybir.dt.float32r
BF16 = mybir.dt.bfloat16


def build_nc(with_bias=False):
    nc = bacc.Bacc("TRN2", target_bir_lowering=False, debug=False)
    AF = mybir.ActivationFunctionType

    hsT_d = nc.dram_tensor("hsT", [NB, H, S], F32R, kind="ExternalInput").ap()
    wqT_d = nc.dram_tensor("wqT", [H, H], F32R, kind="ExternalInput").ap()
    wkT_d = nc.dram_tensor("wkT", [H, H], F32R, kind="ExternalInput").ap()
    wvT_d = nc.dram_tensor("wvT", [H, H], F32R, kind="ExternalInput").ap()
    if with_bias:
        bq_d = nc.dram_tensor("bq", [H], F32, kind="ExternalInput").ap()
        bk_d = nc.dram_tensor("bk", [H], F32, kind="ExternalInput").ap()
        bv_d = nc.dram_tensor("bv", [H], F32, kind="ExternalInput").ap()
    m_d = nc.dram_tensor("madd", [NB, S, S], BF16, kind="ExternalInput").ap()
    gpT_d = nc.dram_tensor("gpT", [NB, S, S], BF16, kind="ExternalInput").ap()
    scores_d = nc.dram_tensor("scores", [NB, NH, S, S], F32, kind="ExternalOutput").ap()
    ctx_d = nc.dram_tensor("ctx", [NB, S, H], F32, kind="ExternalOutput").ap()

    with tile.TileContext(nc) as tc:
        with (
            tc.tile_pool(name="wpool", bufs=1) as wpool,
            tc.tile_pool(name="bpool", bufs=1) as bpool,
            tc.tile_pool(name="perb", bufs=1) as perb,
            tc.tile_pool(name="work", bufs=1) as work,
            tc.tile_pool(name="psA", bufs=4, space="PSUM") as psA,
            tc.tile_pool(name="psT", bufs=2, space="PSUM") as psT,
            tc.tile_pool(name="psC", bufs=2, space="PSUM") as psC,
        ):
            # ---- one-time loads ------------------------------------------------
            wq_sb = wpool.tile([128, HC, H], F32R, tag="wq")
            wk_sb = wpool.tile([128, HC, H], F32R, tag="wk")
            wv_sb = wpool.tile([128, HC, H], F32R, tag="wv")
            nc.sync.dma_start(out=wq_sb, in_=wqT_d.rearrange("(c p) o -> p c o", p=128))
            nc.sync.dma_start(out=wk_sb, in_=wkT_d.rearrange("(c p) o -> p c o", p=128))
            nc.sync.dma_start(out=wv_sb, in_=wvT_d.rearrange("(c p) o -> p c o", p=128))

            ident = bpool.tile([128, 128], F32, tag="ident")
            make_identity(nc, ident)
            ident_bf = bpool.tile([128, 128], BF16, tag="ident_bf")
            nc.vector.tensor_copy(ident_bf, ident)

            if with_bias:
                bq_sb = bpool.tile([128, HC], F32, tag="bq")
                bk_sb = bpool.tile([128, HC], F32, tag="bk")
                nc.sync.dma_start(out=bq_sb, in_=bq_d.rearrange("(c p) -> p c", p=128))
                nc.sync.dma_start(out=bk_sb, in_=bk_d.rearrange("(c p) -> p c", p=128))
                bvb_sb = bpool.tile([128, H], F32, tag="bvb")
                bv_bcast = bass.AP(tensor=bv_d.tensor, offset=bv_d.offset,
                                   ap=[[0, 128]] + list(bv_d.ap))
                nc.sync.dma_start(out=bvb_sb, in_=bv_bcast)

            for b in range(NB):
                # ---- per-batch loads ------------------------------------------
                hsT_sb = perb.tile([128, HC, S], F32R, tag="hsT")
                nc.sync.dma_start(
                    out=hsT_sb, in_=hsT_d[b].rearrange("(c p) s -> p c s", p=128))
                m_sb = perb.tile([128, SB, S], BF16, tag="m")
                nc.sync.dma_start(
                    out=m_sb, in_=m_d[b].rearrange("(r p) j -> p r j", p=128))
                gpT_sb = perb.tile([128, SB, S], BF16, tag="gpT")
                nc.sync.dma_start(
                    out=gpT_sb, in_=gpT_d[b].rearrange("(c p) i -> p c i", p=128))

                # ---- projections ----------------------------------------------
                qT_sb = perb.tile([128, HC, S], F32R, tag="qT")
                kT_sb = perb.tile([128, HC, S], F32R, tag="kT")
                for wi, (w_sb, o_sb) in enumerate(((wq_sb, qT_sb), (wk_sb, kT_sb))):
                    for co in range(HC):
                        ps = psA.tile([128, S], F32, tag="ps", name="ps_proj")
                        for ci in range(HC):
                            nc.tensor.matmul(
                                ps, w_sb[:, ci, co * 128:(co + 1) * 128],
                                hsT_sb[:, ci, :],
                                start=(ci == 0), stop=(ci == HC - 1))
                        if with_bias:
                            b_sb = bq_sb if wi == 0 else bk_sb
                            nc.scalar.activation(
                                o_sb[:, co, :], ps, AF.Identity,
                                bias=b_sb[:, co:co + 1])
                        else:
                            nc.scalar.activation(o_sb[:, co, :], ps, AF.Copy)
                v_sb = perb.tile([128, SB, H], BF16, tag="v")
                for sb_i in range(SB):
                    for half in range(2):
                        ps = psA.tile([128, S], F32, tag="ps", name="ps_v")
                        for ci in range(HC):
                            nc.tensor.matmul(
                                ps, hsT_sb[:, ci, sb_i * 128:(sb_i + 1) * 128],
                                wv_sb[:, ci, half * 512:(half + 1) * 512],
                                start=(ci == 0), stop=(ci == HC - 1))
                        dst = v_sb[:, sb_i, half * 512:(half + 1) * 512]
                        if with_bias:
                            nc.vector.tensor_add(
                                dst, ps, bvb_sb[:, half * 512:(half + 1) * 512])
                        else:
                            nc.vector.tensor_copy(dst, ps)

                # ---- attention ------------------------------------------------
                ctx_sb = perb.tile([128, SB, H], F32, tag="ctx")
                for hpair in range(NH // 2):
                    ctxT_pair = work.tile([128, S], BF16, tag="ctT", bufs=2,
                                          name="ctxT_pair")
                    rs_pair = []
                    for hh in range(2):
                        h = hpair * 2 + hh
                        hp = hh * 64
                        masked = work.tile([128, SB, S], F32, tag="masked", bufs=2,
                                           name="masked")
                        ssum = work.tile([128, SB], F32, tag="ssum", bufs=2,
                                         name="ssum")
                        pgT_full = work.tile([128, SB, S], BF16, tag="pgT", bufs=2,
                                             name="pgT_full")
                        for r in range(SB):
                            ps_s = psA.tile([128, S], F32, tag="ps", name="ps_s")
                            nc.tensor.matmul(
                                ps_s, qT_sb[hp:hp + 64, hpair, r * 128:(r + 1) * 128],
                                kT_sb[hp:hp + 64, hpair, :], start=True, stop=True)
                            nc.vector.tensor_add(masked[:, r, :], ps_s, m_sb[:, r, :])
                            p_sb = work.tile([128, S], BF16, tag="p", bufs=3,
                                             name="p_sb")
                            nc.scalar.activation(p_sb, masked[:, r, :], AF.Exp,
                                                 accum_out=ssum[:, r:r + 1])
                            ps_t = psT.tile([128, S], BF16, tag="pst", name="ps_t")
                            for c in range(SB):
                                nc.tensor.transpose(
                                    ps_t[:, c * 128:(c + 1) * 128],
                                    p_sb[:, c * 128:(c + 1) * 128], ident_bf)
                            nc.vector.tensor_mul(
                                pgT_full[:, :, r * 128:(r + 1) * 128],
                                ps_t.rearrange("p (c i) -> p c i", c=SB),
                                gpT_sb[:, :, r * 128:(r + 1) * 128])
                        nc.sync.dma_start(
                            out=scores_d[b, h].rearrange("(r p) j -> p r j", p=128),
                            in_=masked)
                        ps_ct = psC.tile([64, S], F32, tag="psct", name="ps_ct")
                        for c in range(SB):
                            nc.tensor.matmul(
                                ps_ct, v_sb[:, c, h * DH:(h + 1) * DH],
                                pgT_full[:, c, :], start=(c == 0), stop=(c == SB - 1))
                        nc.vector.tensor_copy(ctxT_pair[hp:hp + 64, :], ps_ct)
                        rs = work.tile([128, SB], F32, tag="rs", bufs=3, name="rs")
                        nc.vector.reciprocal(rs, ssum)
                        rs_pair.append(rs)
                    ps_x = psT.tile([128, S], BF16, tag="pst", name="ps_x")
                    for c in range(SB):
                        nc.tensor.transpose(
                            ps_x[:, c * 128:(c + 1) * 128],
                            ctxT_pair[:, c * 128:(c + 1) * 128], ident_bf)
                    for c in range(SB):
                        for hh in range(2):
                            h = hpair * 2 + hh
                            nc.scalar.mul(
                                ctx_sb[:, c, h * DH:(h + 1) * DH],
                                ps_x[:, c * 128 + hh * 64:c * 128 + hh * 64 + 64],
                                rs_pair[hh][:, c:c + 1])
                nc.sync.dma_start(
                    out=ctx_d[b].rearrange("(r p) o -> p r o", p=128), in_=ctx_sb)
    nc.compile()
    return nc


_NC_CACHE = {}


def _get_nc(with_bias=False):
    key = ("nc", with_bias)
    if key not in _NC_CACHE:
        _NC_CACHE[key] = build_nc(with_bias)
    return _NC_CACHE[key]


def prep_inputs(hidden_states, attention_mask, group_prob, Wq, bq, Wk, bk, Wv, bv):
    """Host-side layout prep + sharding. Returns (in_maps, with_bias)."""
    f = np.float32
    hs = np.asarray(hidden_states, dtype=f)
    hsT = np.ascontiguousarray(hs.transpose(0, 2, 1))
    wqT = np.ascontiguousarray(np.asarray(Wq, dtype=f).T / 8.0)
    wkT = np.ascontiguousarray(np.asarray(Wk, dtype=f).T)
    wvT = np.ascontiguousarray(np.asarray(Wv, dtype=f).T)
    bq8 = np.asarray(bq, dtype=f) / 8.0
    bk_ = np.asarray(bk, dtype=f)
    bv_ = np.asarray(bv, dtype=f)
    with_bias = bool(np.abs(bq8).max() or np.abs(bk_).max() or np.abs(bv_).max())
    keep = (np.asarray(attention_mask)[:, 0] != 0) | np.eye(S, dtype=bool)
    madd = np.where(keep, 0, -np.inf).astype(ml_dtypes.bfloat16)
    gpT = np.ascontiguousarray(
        np.asarray(group_prob, dtype=f).transpose(0, 2, 1)).astype(ml_dtypes.bfloat16)
    in_maps = []
    for i in range(NCORES):
        sl = slice(i * NB, (i + 1) * NB)
        m = {
            "hsT": hsT[sl], "wqT": wqT, "wkT": wkT, "wvT": wvT,
            "madd": madd[sl], "gpT": gpT[sl],
        }
        if with_bias:
            m.update({"bq": bq8, "bk": bk_, "bv": bv_})
        in_maps.append(m)
    return in_maps, with_bias


def kernel(hidden_states, attention_mask, group_prob, Wq, bq, Wk, bk, Wv, bv):
    in_maps, with_bias = prep_inputs(hidden_states, attention_mask, group_prob,
                                     Wq, bq, Wk, bk, Wv, bv)
    nc = _get_nc(with_bias)
    res = run_bass_kernel_spmd(nc, in_maps, core_ids=list(range(NCORES)))
    ctx = np.concatenate([res.results[i]["ctx"] for i in range(NCORES)], axis=0)
    scores = np.concatenate([res.results[i]["scores"] for i in range(NCORES)], axis=0)
    return ctx, scores


# revision 6
# speedup vs baseline: 1.1488x; 1.1488x over previous
"""BertSelfAttention (group_prob-scaled probs, mask|diag masking) on 8 TRN2 cores.

Sharding: data-parallel over batch (16 -> 2 per core). Device math per (b, head):
  v = hs @ Wv^T  first, then per hidden-chunk co: qT/kT chunk projections
  (f32r matmuls) immediately followed by attention for heads 2co, 2co+1:
    scores = qT_h^T @ kT_h   (1/8 folded into Wq on host)
    masked = scores + M      (M in {0, -inf}; exact -inf via DVE add)
    p = exp(masked) [bf16] with accumulated row-sum (ACT)
    pgT = transpose(p) * gpT [bf16]  (PE transpose + fused DVE multiply)
    ctx_h = (pg^T)^T @ v_h / rowsum  (bf16 matmuls, fp32 accumulate)
Host side is layout-only: transposes (hs^T, W^T, gp^T), 1/8 scale fold (exact,
power of two), additive mask build, bf16 casts for the probability path.
"""

import os
import sys

import numpy as np

for _p in ("/opt/trn_rl_repo", "/root/.axon_site/_ro/trn_rl_repo"):
    if _p not in sys.path and os.path.isdir(_p):
        sys.path.append(_p)

import ml_dtypes
import concourse.bacc as bacc
import concourse.bass as bass
import concourse.tile as tile
from concourse import mybir
from concourse.bass_utils import run_bass_kernel_spmd
from concourse.masks import make_identity

NB = 2          # batches per core
S = 512         # sequence length
H = 1024        # hidden
NH = 16         # heads
DH = 64         # head dim
NCORES = 8
HC = H // 128   # 8 hidden chunks
SB = S // 128   # 4 seq blocks

F32 = mybir.dt.float32
F32R = mybir.dt.float32r
BF16 = mybir.dt.bfloat16


def build_nc(with_bias=False):
    nc = bacc.Bacc("TRN2", target_bir_lowering=False, debug=False)
    AF = mybir.ActivationFunctionType

    hsT_d = nc.dram_tensor("hsT", [NB, H, S], F32R, kind="ExternalInput").ap()
    wqT_d = nc.dram_tensor("wqT", [H, H], F32R, kind="ExternalInput").ap()
    wkT_d = nc.dram_tensor("wkT", [H, H], F32R, kind="ExternalInput").ap()
    wvT_d = nc.dram_tensor("wvT", [H, H], F32R, kind="ExternalInput").ap()
    if with_bias:
        bq_d = nc.dram_tensor("bq", [H], F32, kind="ExternalInput").ap()
        bk_d = nc.dram_tensor("bk", [H], F32, kind="ExternalInput").ap()
        bv_d = nc.dram_tensor("bv", [H], F32, kind="ExternalInput").ap()
    m_d = nc.dram_tensor("madd", [NB, S, S], BF16, kind="ExternalInput").ap()
    gpT_d = nc.dram_tensor("gpT", [NB, S, S], BF16, kind="ExternalInput").ap()
    scores_d = nc.dram_tensor("scores", [NB, NH, S, S], F32, kind="ExternalOutput").ap()
    ctx_d = nc.dram_tensor("ctx", [NB, S, H], F32, kind="ExternalOutput").ap()

    with tile.TileContext(nc) as tc:
        with (
            tc.tile_pool(name="wpool", bufs=1) as wpool,
            tc.tile_pool(name="bpool", bufs=1) as bpool,
            tc.tile_pool(name="perb", bufs=1) as perb,
            tc.tile_pool(name="work", bufs=1) as work,
            tc.tile_pool(name="psA", bufs=4, space="PSUM") as psA,
            tc.tile_pool(name="psT", bufs=2, space="PSUM") as psT,
            tc.tile_pool(name="psC", bufs=2, space="PSUM") as psC,
        ):
            # ---- one-time loads ------------------------------------------------
            wq_sb = wpool.tile([128, HC, H], F32R, tag="wq")
            wk_sb = wpool.tile([128, HC, H], F32R, tag="wk")
            wv_sb = wpool.tile([128, HC, H], F32R, tag="wv")
            nc.sync.dma_start(out=wq_sb, in_=wqT_d.rearrange("(c p) o -> p c o", p=128))
            nc.sync.dma_start(out=wk_sb, in_=wkT_d.rearrange("(c p) o -> p c o", p=128))
            nc.sync.dma_start(out=wv_sb, in_=wvT_d.rearrange("(c p) o -> p c o", p=128))

            ident = bpool.tile([128, 128], F32, tag="ident")
            make_identity(nc, ident)
            ident_bf = bpool.tile([128, 128], BF16, tag="ident_bf")
            nc.vector.tensor_copy(ident_bf, ident)

            if with_bias:
                bq_sb = bpool.tile([128, HC], F32, tag="bq")
                bk_sb = bpool.tile([128, HC], F32, tag="bk")
                nc.sync.dma_start(out=bq_sb, in_=bq_d.rearrange("(c p) -> p c", p=128))
                nc.sync.dma_start(out=bk_sb, in_=bk_d.rearrange("(c p) -> p c", p=128))
                bvb_sb = bpool.tile([128, H], F32, tag="bvb")
                bv_bcast = bass.AP(tensor=bv_d.tensor, offset=bv_d.offset,
                                   ap=[[0, 128]] + list(bv_d.ap))
                nc.sync.dma_start(out=bvb_sb, in_=bv_bcast)

            for b in range(NB):
                # ---- per-batch loads ------------------------------------------
                hsT_sb = perb.tile([128, HC, S], F32R, tag="hsT")
                nc.sync.dma_start(
                    out=hsT_sb, in_=hsT_d[b].rearrange("(c p) s -> p c s", p=128))
                m_sb = perb.tile([128, SB, S], BF16, tag="m")
                nc.sync.dma_start(
                    out=m_sb, in_=m_d[b].rearrange("(r p) j -> p r j", p=128))
                gpT_sb = perb.tile([128, SB, S], BF16, tag="gpT")
                nc.sync.dma_start(
                    out=gpT_sb, in_=gpT_d[b].rearrange("(c p) i -> p c i", p=128))

                # ---- v projection ---------------------------------------------
                v_sb = perb.tile([128, SB, H], BF16, tag="v")
                for sb_i in range(SB):
                    for half in range(2):
                        ps = psA.tile([128, S], F32, tag="ps", name="ps_v")
                        for ci in range(HC):
                            nc.tensor.matmul(
                                ps, hsT_sb[:, ci, sb_i * 128:(sb_i + 1) * 128],
                                wv_sb[:, ci, half * 512:(half + 1) * 512],
                                start=(ci == 0), stop=(ci == HC - 1))
                        dst = v_sb[:, sb_i, half * 512:(half + 1) * 512]
                        if with_bias:
                            nc.vector.tensor_add(
                                dst, ps, bvb_sb[:, half * 512:(half + 1) * 512])
                        else:
                            nc.vector.tensor_copy(dst, ps)

                # ---- q/k chunk projections interleaved with attention ----------
                ctx_sb = perb.tile([128, SB, H], F32, tag="ctx")
                for co in range(HC):
                    qT_c = work.tile([128, S], F32R, tag="qTc", bufs=3, name="qT_c")
                    kT_c = work.tile([128, S], F32R, tag="kTc", bufs=3, name="kT_c")
                    for wi, (w_sb, o_sb) in enumerate(((wq_sb, qT_c), (wk_sb, kT_c))):
                        ps = psA.tile([128, S], F32, tag="ps", name="ps_proj")
                        for ci in range(HC):
                            nc.tensor.matmul(
                                ps, w_sb[:, ci, co * 128:(co + 1) * 128],
                                hsT_sb[:, ci, :],
                                start=(ci == 0), stop=(ci == HC - 1))
                        if with_bias:
                            b_sb = bq_sb if wi == 0 else bk_sb
                            nc.scalar.activation(o_sb, ps, AF.Identity,
                                                 bias=b_sb[:, co:co + 1])
                        else:
                            nc.scalar.activation(o_sb, ps, AF.Copy)

                    for hh in range(2):
                        h = co * 2 + hh
                        hp = hh * 64
                        masked = work.tile([128, SB, S], F32, tag="masked", bufs=3,
                                           name="masked")
                        ssum = work.tile([128, SB], F32, tag="ssum", bufs=2,
                                         name="ssum")
                        ps_c = psC.tile([128, SB, DH], F32, tag="psc", name="ps_c")
                        for r in range(SB):
                            ps_s = psA.tile([128, S], F32, tag="ps", name="ps_s")
                            nc.tensor.matmul(
                                ps_s, qT_c[hp:hp + 64, r * 128:(r + 1) * 128],
                                kT_c[hp:hp + 64, :], start=True, stop=True)
                            nc.vector.tensor_add(masked[:, r, :], ps_s, m_sb[:, r, :])
                            p_sb = work.tile([128, S], BF16, tag="p", bufs=4,
                                             name="p_sb")
                            nc.scalar.activation(p_sb, masked[:, r, :], AF.Exp,
                                                 accum_out=ssum[:, r:r + 1])
                            ps_t = psT.tile([128, S], BF16, tag="pst", name="ps_t")
                            for c in range(SB):
                                nc.tensor.transpose(
                                    ps_t[:, c * 128:(c + 1) * 128],
                                    p_sb[:, c * 128:(c + 1) * 128], ident_bf)
                            pgT = work.tile([128, SB, 128], BF16, tag="pgT", bufs=4,
                                            name="pgT")
                            nc.vector.tensor_mul(
                                pgT, ps_t.rearrange("p (c i) -> p c i", c=SB),
                                gpT_sb[:, :, r * 128:(r + 1) * 128])
                            for c in range(SB):
                                nc.tensor.matmul(
                                    ps_c[:, r, :], pgT[:, c, :],
                                    v_sb[:, c, h * DH:(h + 1) * DH],
                                    start=(c == 0), stop=(c == SB - 1))
                        nc.sync.dma_start(
                            out=scores_d[b, h].rearrange("(r p) j -> p r j", p=128),
                            in_=masked)
                        rs = work.tile([128, SB], F32, tag="rs", bufs=2, name="rs")
                        nc.vector.reciprocal(rs, ssum)
                        for r in range(SB):
                            if hh == 0:
                                nc.scalar.mul(ctx_sb[:, r, h * DH:(h + 1) * DH],
                                              ps_c[:, r, :], rs[:, r:r + 1])
                            else:
                                nc.vector.tensor_scalar(
                                    out=ctx_sb[:, r, h * DH:(h + 1) * DH],
                                    in0=ps_c[:, r, :], scalar1=rs[:, r:r + 1],
                                    scalar2=None, op0=mybir.AluOpType.mult)
                nc.sync.dma_start(
                    out=ctx_d[b].rearrange("(r p) o -> p r o", p=128), in_=ctx_sb)
    nc.compile()
    return nc


_NC_CACHE = {}


def _get_nc(with_bias=False):
    key = ("nc", with_bias)
    if key not in _NC_CACHE:
        _NC_CACHE[key] = build_nc(with_bias)
    return _NC_CACHE[key]


def prep_inputs(hidden_states, attention_mask, group_prob, Wq, bq, Wk, bk, Wv, bv):
    """Host-side layout prep + sharding. Returns (in_maps, with_bias)."""
    f = np.float32
    hs = np.asarray(hidden_states, dtype=f)
    hsT = np.ascontiguousarray(hs.transpose(0, 2, 1))
    wqT = np.ascontiguousarray(np.asarray(Wq, dtype=f).T / 8.0)
    wkT = np.ascontiguousarray(np.asarray(Wk, dtype=f).T)
    wvT = np.ascontiguousarray(np.asarray(Wv, dtype=f).T)
    bq8 = np.asarray(bq, dtype=f) / 8.0
    bk_ = np.asarray(bk, dtype=f)
    bv_ = np.asarray(bv, dtype=f)
    with_bias = bool(np.abs(bq8).max() or np.abs(bk_).max() or np.abs(bv_).max())
    keep = (np.asarray(attention_mask)[:, 0] != 0) | np.eye(S, dtype=bool)
    madd = np.where(keep, 0, -np.inf).astype(ml_dtypes.bfloat16)
    gpT = np.ascontiguousarray(
        np.asarray(group_prob, dtype=f).transpose(0, 2, 1)).astype(ml_dtypes.bfloat16)
    in_maps = []
    for i in range(NCORES):
        sl = slice(i * NB, (i + 1) * NB)
        m = {
            "hsT": hsT[sl], "wqT": wqT, "wkT": wkT, "wvT": wvT,
            "madd": madd[sl], "gpT": gpT[sl],
        }
        if with_bias:
            m.update({"bq": bq8, "bk": bk_, "bv": bv_})
        in_maps.append(m)
    return in_maps, with_bias


def kernel(hidden_states, attention_mask, group_prob, Wq, bq, Wk, bk, Wv, bv):
    in_maps, with_bias = prep_inputs(hidden_states, attention_mask, group_prob,
                                     Wq, bq, Wk, bk, Wv, bv)
    nc = _get_nc(with_bias)
    res = run_bass_kernel_spmd(nc, in_maps, core_ids=list(range(NCORES)))
    ctx = np.concatenate([res.results[i]["ctx"] for i in range(NCORES)], axis=0)
    scores = np.concatenate([res.results[i]["scores"] for i in range(NCORES)], axis=0)
    return ctx, scores


# revision 15
# speedup vs baseline: 1.2902x; 1.1231x over previous
"""BertSelfAttention (group_prob-scaled probs, mask|diag masking) on 8 TRN2 cores.

Sharding: data-parallel over batch (16 -> 2 per core). Device math per (b, head):
  v = hs @ Wv^T  first, then per hidden-chunk co: qT/kT chunk projections
  (f32r matmuls) immediately followed by attention for heads 2co, 2co+1:
    scores = qT_h^T @ kT_h   (1/8 folded into Wq on host)
    masked = scores + M      (M in {0, -inf}; exact -inf via DVE add)
    p = exp(masked) [bf16] with accumulated row-sum (ACT)
    pgT = transpose(p) * gpT [bf16]  (PE transpose + fused DVE multiply)
    ctx_h = (pg^T)^T @ v_h / rowsum  (bf16 matmuls, fp32 accumulate)
Emission is hand-scheduled: chunked input DMAs on the ACT HWDGE queue so
compute starts after ~1MB, outputs on the SP queue, and batch 1's v/projection
work interleaved into batch 0's attention tail to avoid pipeline bubbles.
Host side is layout-only: transposes (hs^T, W^T, gp^T), 1/8 scale fold (exact,
power of two), additive mask build, bf16 casts for the probability path.
"""

import os
import sys

import numpy as np

for _p in ("/opt/trn_rl_repo", "/root/.axon_site/_ro/trn_rl_repo"):
    if _p not in sys.path and os.path.isdir(_p):
        sys.path.append(_p)

import ml_dtypes
import concourse.bacc as bacc
import concourse.bass as bass
import concourse.tile as tile
from concourse import mybir
from concourse.bass_utils import run_bass_kernel_spmd
from concourse.masks import make_identity

NB = 2          # batches per core
S = 512         # sequence length
H = 1024        # hidden
NH = 16         # heads
DH = 64         # head dim
NCORES = 8
HC = H // 128   # 8 hidden chunks
SB = S // 128   # 4 seq blocks

F32 = mybir.dt.float32
F32R = mybir.dt.float32r
BF16 = mybir.dt.bfloat16


SCORES_BF16 = False


def build_nc(with_bias=False):
    nc = bacc.Bacc("TRN2", target_bir_lowering=False, debug=False)
    AF = mybir.ActivationFunctionType

    hsT_d = nc.dram_tensor("hsT", [NB, H, S], F32R, kind="ExternalInput").ap()
    wqT_d = nc.dram_tensor("wqT", [H, H], F32R, kind="ExternalInput").ap()
    wkT_d = nc.dram_tensor("wkT", [H, H], F32R, kind="ExternalInput").ap()
    wvT_d = nc.dram_tensor("wvT", [H, H], F32R, kind="ExternalInput").ap()
    if with_bias:
        bq_d = nc.dram_tensor("bq", [H], F32, kind="ExternalInput").ap()
        bk_d = nc.dram_tensor("bk", [H], F32, kind="ExternalInput").ap()
        bv_d = nc.dram_tensor("bv", [H], F32, kind="ExternalInput").ap()
    m_d = nc.dram_tensor("madd", [NB, S, S], BF16, kind="ExternalInput").ap()
    gpT_d = nc.dram_tensor("gpT", [NB, S, S], BF16, kind="ExternalInput").ap()
    sdt = BF16 if SCORES_BF16 else F32
    scores_d = nc.dram_tensor("scores", [NB, NH, S, S], sdt,
                              kind="ExternalOutput").ap()
    ctx_d = nc.dram_tensor("ctx", [NB, S, H], F32, kind="ExternalOutput").ap()

    with tile.TileContext(nc) as tc:
        with (
            tc.tile_pool(name="wpool", bufs=1) as wpool,
            tc.tile_pool(name="bpool", bufs=1) as bpool,
            tc.tile_pool(name="perb", bufs=1) as perb,
            tc.tile_pool(name="work", bufs=1) as work,
            tc.tile_pool(name="psA", bufs=3, space="PSUM") as psA,
            tc.tile_pool(name="psT", bufs=3, space="PSUM") as psT,
            tc.tile_pool(name="psC", bufs=2, space="PSUM") as psC,
        ):
            # ---- DMA plumbing: per-chunk tiles, inputs split across queues -----
            wqT_r = wqT_d.rearrange("(c p) o -> p c o", p=128)
            wkT_r = wkT_d.rearrange("(c p) o -> p c o", p=128)
            wvT_r = wvT_d.rearrange("(c p) o -> p c o", p=128)

            wv_cis, wq_cos, wk_cos = [], [], []
            for ci in range(HC):
                t = wpool.tile([128, H], F32R, tag=f"wv{ci}", name=f"wv_ci{ci}")
                wv_cis.append(t)
            for co in range(HC):
                tq = wpool.tile([128, HC, 128], F32R, tag=f"wq{co}",
                                name=f"wq_co{co}")
                tk = wpool.tile([128, HC, 128], F32R, tag=f"wk{co}",
                                name=f"wk_co{co}")
                wq_cos.append(tq)
                wk_cos.append(tk)

            def load_hsT(b, queue):
                ts = []
                hsT_r = hsT_d[b].rearrange("(c p) s -> p c s", p=128)
                for ci in range(HC):
                    t = perb.tile([128, S], F32R, tag=f"hsT{ci}", bufs=2,
                                  name=f"hsT_b{b}c{ci}")
                    queue.dma_start(out=t, in_=hsT_r[:, ci, :])
                    ts.append(t)
                return ts

            def load_mgp(b):
                mt = perb.tile([128, SB, S], BF16, tag="m", name=f"m_sb{b}")
                nc.scalar.dma_start(
                    out=mt, in_=m_d[b].rearrange("(r p) j -> p r j", p=128))
                gt = perb.tile([128, SB, S], BF16, tag="gpT", name=f"gpT_sb{b}")
                nc.scalar.dma_start(
                    out=gt, in_=gpT_d[b].rearrange("(c p) i -> p c i", p=128))
                return mt, gt

            # scalar queue: wv + hsT0 + m/gp; sync queue: wq/wk + hsT1
            hsT0 = []
            for ci in range(HC):
                nc.scalar.dma_start(out=wv_cis[ci], in_=wvT_r[:, ci, :])
                t = perb.tile([128, S], F32R, tag=f"hsT{ci}", bufs=2,
                              name=f"hsT_b0c{ci}")
                nc.scalar.dma_start(
                    out=t,
                    in_=hsT_d[0].rearrange("(c p) s -> p c s", p=128)[:, ci, :])
                hsT0.append(t)
            mgp0 = load_mgp(0)
            for co in range(HC):
                sl = slice(co * 128, (co + 1) * 128)
                nc.gpsimd.dma_start(out=wq_cos[co], in_=wqT_r[:, :, sl])
                nc.gpsimd.dma_start(out=wk_cos[co], in_=wkT_r[:, :, sl])
            hsT1 = load_hsT(1, nc.gpsimd)

            ident = bpool.tile([128, 128], F32, tag="ident")
            make_identity(nc, ident)
            ident_bf = bpool.tile([128, 128], BF16, tag="ident_bf")
            nc.vector.tensor_copy(ident_bf, ident)

            if with_bias:
                bq_sb = bpool.tile([128, HC], F32, tag="bq")
                bk_sb = bpool.tile([128, HC], F32, tag="bk")
                nc.sync.dma_start(out=bq_sb, in_=bq_d.rearrange("(c p) -> p c", p=128))
                nc.sync.dma_start(out=bk_sb, in_=bk_d.rearrange("(c p) -> p c", p=128))
                bvb_sb = bpool.tile([128, H], F32, tag="bvb")
                bv_bcast = bass.AP(tensor=bv_d.tensor, offset=bv_d.offset,
                                   ap=[[0, 128]] + list(bv_d.ap))
                nc.sync.dma_start(out=bvb_sb, in_=bv_bcast)

            hsTs = {0: hsT0, 1: hsT1}
            mgps = {0: mgp0}
            v_sbs = {}

            def alloc_v(b):
                v_sbs[b] = perb.tile([128, SB, H], BF16, tag="v", bufs=2,
                                     name=f"v_sb{b}")

            def v_proj(b, sb_i, half):
                ps = psA.tile([128, S], F32, tag="ps", name="ps_v")
                for ci in range(HC):
                    nc.tensor.matmul(
                        ps, hsTs[b][ci][:, sb_i * 128:(sb_i + 1) * 128],
                        wv_cis[ci][:, half * 512:(half + 1) * 512],
                        start=(ci == 0), stop=(ci == HC - 1))
                dst = v_sbs[b][:, sb_i, half * 512:(half + 1) * 512]
                if with_bias:
                    nc.vector.tensor_add(
                        dst, ps, bvb_sb[:, half * 512:(half + 1) * 512])
                else:
                    nc.vector.tensor_copy(dst, ps)

            def emit_co(b, co):
                hsT_sb = hsTs[b]
                m_sb, gpT_sb = mgps[b]
                v_sb = v_sbs[b]
                ctx_sb = work.tile([128, SB, 2 * DH], F32, tag="ctxco", bufs=3,
                                   name="ctx_co")
                qT_c = work.tile([128, S], F32R, tag="qTc", bufs=3, name="qT_c")
                kT_c = work.tile([128, S], F32R, tag="kTc", bufs=3, name="kT_c")
                for wi, (w_sb, o_sb) in enumerate(
                        ((wq_cos[co], qT_c), (wk_cos[co], kT_c))):
                    ps = psA.tile([128, S], F32, tag="ps", name="ps_proj")
                    for ci in range(HC):
                        nc.tensor.matmul(
                            ps, w_sb[:, ci, :], hsT_sb[ci],
                            start=(ci == 0), stop=(ci == HC - 1))
                    if with_bias:
                        b_sb = bq_sb if wi == 0 else bk_sb
                        nc.scalar.activation(o_sb, ps, AF.Identity,
                                             bias=b_sb[:, co:co + 1])
                    else:
                        nc.scalar.activation(o_sb, ps, AF.Copy)

                for hh in range(2):
                    h = co * 2 + hh
                    hp = hh * 64
                    masked = work.tile([128, SB, S], sdt, tag="masked", bufs=3,
                                       name="masked")
                    ssum = work.tile([128, SB], F32, tag="ssum", bufs=2, name="ssum")
                    ps_c = psC.tile([128, SB, DH], F32, tag="psc", name="ps_c")
                    for r in range(SB):
                        ps_s = psA.tile([128, S], F32, tag="ps", name="ps_s")
                        nc.tensor.matmul(
                            ps_s, qT_c[hp:hp + 64, r * 128:(r + 1) * 128],
                            kT_c[hp:hp + 64, :], start=True, stop=True)
                        nc.vector.tensor_add(masked[:, r, :], ps_s, m_sb[:, r, :])
                        p_sb = work.tile([128, S], BF16, tag="p", bufs=6, name="p_sb")
                        nc.scalar.activation(p_sb, masked[:, r, :], AF.Exp,
                                             accum_out=ssum[:, r:r + 1])
                        ps_t = psT.tile([128, S], BF16, tag="pst", name="ps_t")
                        for c in range(SB):
                            nc.tensor.transpose(
                                ps_t[:, c * 128:(c + 1) * 128],
                                p_sb[:, c * 128:(c + 1) * 128], ident_bf)
                        pgT = work.tile([128, SB, 128], BF16, tag="pgT", bufs=6,
                                        name="pgT")
                        nc.vector.tensor_mul(
                            pgT, ps_t.rearrange("p (c i) -> p c i", c=SB),
                            gpT_sb[:, :, r * 128:(r + 1) * 128])
                        for c in range(SB):
                            nc.tensor.matmul(
                                ps_c[:, r, :], pgT[:, c, :],
                                v_sb[:, c, h * DH:(h + 1) * DH],
                                start=(c == 0), stop=(c == SB - 1))
                    nc.sync.dma_start(
                        out=scores_d[b, h].rearrange("(r p) j -> p r j", p=128),
                        in_=masked)
                    rs = work.tile([128, SB], F32, tag="rs", bufs=2, name="rs")
                    nc.vector.reciprocal(rs, ssum)
                    for r in range(SB):
                        if hh == 0:
                            nc.scalar.mul(ctx_sb[:, r, hh * DH:(hh + 1) * DH],
                                          ps_c[:, r, :], rs[:, r:r + 1])
                        else:
                            nc.vector.tensor_scalar(
                                out=ctx_sb[:, r, hh * DH:(hh + 1) * DH],
                                in0=ps_c[:, r, :], scalar1=rs[:, r:r + 1],
                                scalar2=None, op0=mybir.AluOpType.mult)
                csl = slice(co * 2 * DH, (co + 1) * 2 * DH)
                nc.sync.dma_start(
                    out=ctx_d[b].rearrange("(r p) o -> p r o", p=128)[:, :, csl],
                    in_=ctx_sb)

            # ---- hand-scheduled emission --------------------------------------
            alloc_v(0)
            for sb_i in range(SB):
                v_proj(0, sb_i, 0)
            for sb_i in range(SB):
                v_proj(0, sb_i, 1)
            for co in range(5):
                emit_co(0, co)
            mgps[1] = load_mgp(1)
            alloc_v(1)
            v_proj(1, 0, 0)
            emit_co(0, 5)
            v_proj(1, 1, 0)
            emit_co(0, 6)
            v_proj(1, 2, 0)
            emit_co(0, 7)
            v_proj(1, 3, 0)
            for co in range(SB):
                v_proj(1, co, 1)
                emit_co(1, co)
            for co in range(SB, HC):
                emit_co(1, co)
    nc.compile()
    return nc


_NC_CACHE = {}


def _get_nc(with_bias=False):
    key = ("nc", with_bias)
    if key not in _NC_CACHE:
        _NC_CACHE[key] = build_nc(with_bias)
    return _NC_CACHE[key]


def prep_inputs(hidden_states, attention_mask, group_prob, Wq, bq, Wk, bk, Wv, bv):
    """Host-side layout prep + sharding. Returns (in_maps, with_bias)."""
    f = np.float32
    hs = np.asarray(hidden_states, dtype=f)
    hsT = np.ascontiguousarray(hs.transpose(0, 2, 1))
    wqT = np.ascontiguousarray(np.asarray(Wq, dtype=f).T / 8.0)
    wkT = np.ascontiguousarray(np.asarray(Wk, dtype=f).T)
    wvT = np.ascontiguousarray(np.asarray(Wv, dtype=f).T)
    bq8 = np.asarray(bq, dtype=f) / 8.0
    bk_ = np.asarray(bk, dtype=f)
    bv_ = np.asarray(bv, dtype=f)
    with_bias = bool(np.abs(bq8).max() or np.abs(bk_).max() or np.abs(bv_).max())
    keep = (np.asarray(attention_mask)[:, 0] != 0) | np.eye(S, dtype=bool)
    madd = np.where(keep, 0, -np.inf).astype(ml_dtypes.bfloat16)
    gpT = np.ascontiguousarray(
        np.asarray(group_prob, dtype=f).transpose(0, 2, 1)).astype(ml_dtypes.bfloat16)
    in_maps = []
    for i in range(NCORES):
        sl = slice(i * NB, (i + 1) * NB)
        m = {
            "hsT": hsT[sl], "wqT": wqT, "wkT": wkT, "wvT": wvT,
            "madd": madd[sl], "gpT": gpT[sl],
        }
        if with_bias:
            m.update({"bq": bq8, "bk": bk_, "bv": bv_})
        in_maps.append(m)
    return in_maps, with_bias


def kernel(hidden_states, attention_mask, group_prob, Wq, bq, Wk, bk, Wv, bv):
    in_maps, with_bias = prep_inputs(hidden_states, attention_mask, group_prob,
                                     Wq, bq, Wk, bk, Wv, bv)
    nc = _get_nc(with_bias)
    res = None
    last_err = None
    for _attempt in range(3):
        try:
            res = run_bass_kernel_spmd(nc, in_maps, core_ids=list(range(NCORES)))
            break
        except Exception as e:  # transient NRT device errors: retry
            last_err = e
    if res is None:
        raise last_err
    ctx = np.concatenate([res.results[i]["ctx"] for i in range(NCORES)], axis=0)
    scores = np.concatenate(
        [np.asarray(res.results[i]["scores"], dtype=np.float32)
         for i in range(NCORES)], axis=0)
    return ctx, scores


# revision 19
# speedup vs baseline: 1.3928x; 1.0796x over previous
"""BertSelfAttention (group_prob-scaled probs, mask|diag masking) on 8 TRN2 cores.

Sharding: data-parallel over batch (16 -> 2 per core). Device math per (b, head):
  v = hs @ Wv^T  first, then per hidden-chunk co: qT/kT chunk projections
  (f32r matmuls) immediately followed by attention for heads 2co, 2co+1:
    scores = qT_h^T @ kT_h   (1/8 folded into Wq on host)
    masked = scores + M      (M in {0, -inf}; exact -inf via DVE add)
    p = exp(masked) [bf16] with accumulated row-sum (ACT)
    pgT = transpose(p) * gpT [bf16]  (PE transpose + fused DVE multiply)
    ctx_h = (pg^T)^T @ v_h / rowsum  (bf16 matmuls, fp32 accumulate)
Emission is hand-scheduled: chunked input DMAs on the ACT HWDGE queue so
compute starts after ~1MB, outputs on the SP queue, and batch 1's v/projection
work interleaved into batch 0's attention tail to avoid pipeline bubbles.
Host side is layout-only: transposes (hs^T, W^T, gp^T), 1/8 scale fold (exact,
power of two), additive mask build, bf16 casts for the probability path.
"""

import os
import sys

import numpy as np

for _p in ("/opt/trn_rl_repo", "/root/.axon_site/_ro/trn_rl_repo"):
    if _p not in sys.path and os.path.isdir(_p):
        sys.path.append(_p)

import ml_dtypes
import concourse.bacc as bacc
import concourse.bass as bass
import concourse.tile as tile
from concourse import mybir
from concourse.bass_utils import run_bass_kernel_spmd
from concourse.masks import make_identity

NB = 2          # batches per core
S = 512         # sequence length
H = 1024        # hidden
NH = 16         # heads
DH = 64         # head dim
NCORES = 8
HC = H // 128   # 8 hidden chunks
SB = S // 128   # 4 seq blocks

F32 = mybir.dt.float32
F32R = mybir.dt.float32r
BF16 = mybir.dt.bfloat16


SCORES_BF16 = False


def build_nc(with_bias=False):
    nc = bacc.Bacc("TRN2", target_bir_lowering=False, debug=False)
    AF = mybir.ActivationFunctionType

    hsT_d = nc.dram_tensor("hsT", [NB, H, S], F32R, kind="ExternalInput").ap()
    wqT_d = nc.dram_tensor("wqT", [H, H], F32R, kind="ExternalInput").ap()
    wkT_d = nc.dram_tensor("wkT", [H, H], F32R, kind="ExternalInput").ap()
    wvT_d = nc.dram_tensor("wvT", [H, H], F32R, kind="ExternalInput").ap()
    if with_bias:
        bq_d = nc.dram_tensor("bq", [H], F32, kind="ExternalInput").ap()
        bk_d = nc.dram_tensor("bk", [H], F32, kind="ExternalInput").ap()
        bv_d = nc.dram_tensor("bv", [H], F32, kind="ExternalInput").ap()
    m_d = nc.dram_tensor("madd", [NB, S, S], BF16, kind="ExternalInput").ap()
    gpT_d = nc.dram_tensor("gpT", [NB, S, S], BF16, kind="ExternalInput").ap()
    sdt = BF16 if SCORES_BF16 else F32
    scores_d = nc.dram_tensor("scores", [NB, NH, S, S], sdt,
                              kind="ExternalOutput").ap()
    ctx_d = nc.dram_tensor("ctx", [NB, S, H], F32, kind="ExternalOutput").ap()

    with tile.TileContext(nc) as tc:
        with (
            tc.tile_pool(name="wpool", bufs=1) as wpool,
            tc.tile_pool(name="bpool", bufs=1) as bpool,
            tc.tile_pool(name="perb", bufs=1) as perb,
            tc.tile_pool(name="work", bufs=1) as work,
            tc.tile_pool(name="psA", bufs=4, space="PSUM") as psA,
            tc.tile_pool(name="psT", bufs=2, space="PSUM") as psT,
            tc.tile_pool(name="psC", bufs=2, space="PSUM") as psC,
        ):
            # ---- DMA plumbing: per-chunk tiles, inputs split across queues -----
            wqT_r = wqT_d.rearrange("(c p) o -> p c o", p=128)
            wkT_r = wkT_d.rearrange("(c p) o -> p c o", p=128)
            wvT_r = wvT_d.rearrange("(c p) o -> p c o", p=128)

            wv_cis, wq_cos, wk_cos = [], [], []
            for ci in range(HC):
                t = wpool.tile([128, H], F32R, tag=f"wv{ci}", name=f"wv_ci{ci}")
                wv_cis.append(t)
            for co in range(HC):
                tq = wpool.tile([128, HC, 128], F32R, tag=f"wq{co}",
                                name=f"wq_co{co}")
                tk = wpool.tile([128, HC, 128], F32R, tag=f"wk{co}",
                                name=f"wk_co{co}")
                wq_cos.append(tq)
                wk_cos.append(tk)

            def load_hsT(b, queue):
                ts = []
                hsT_r = hsT_d[b].rearrange("(c p) s -> p c s", p=128)
                for ci in range(HC):
                    t = perb.tile([128, S], F32R, tag=f"hsT{ci}", bufs=2,
                                  name=f"hsT_b{b}c{ci}")
                    queue.dma_start(out=t, in_=hsT_r[:, ci, :])
                    ts.append(t)
                return ts

            def load_mgp(b):
                mt = perb.tile([128, SB, S], BF16, tag="m", name=f"m_sb{b}")
                nc.scalar.dma_start(
                    out=mt, in_=m_d[b].rearrange("(r p) j -> p r j", p=128))
                gt = perb.tile([128, SB, S], BF16, tag="gpT", name=f"gpT_sb{b}")
                nc.scalar.dma_start(
                    out=gt, in_=gpT_d[b].rearrange("(c p) i -> p c i", p=128))
                return mt, gt

            # scalar queue: wv + hsT0 + m/gp; sync queue: wq/wk + hsT1
            hsT0 = []
            for ci in range(HC):
                nc.scalar.dma_start(out=wv_cis[ci], in_=wvT_r[:, ci, :])
                t = perb.tile([128, S], F32R, tag=f"hsT{ci}", bufs=2,
                              name=f"hsT_b0c{ci}")
                nc.scalar.dma_start(
                    out=t,
                    in_=hsT_d[0].rearrange("(c p) s -> p c s", p=128)[:, ci, :])
                hsT0.append(t)
            mgp0 = load_mgp(0)
            for co in range(HC):
                sl = slice(co * 128, (co + 1) * 128)
                nc.gpsimd.dma_start(out=wq_cos[co], in_=wqT_r[:, :, sl])
                nc.gpsimd.dma_start(out=wk_cos[co], in_=wkT_r[:, :, sl])
            hsT1 = load_hsT(1, nc.gpsimd)

            ident = bpool.tile([128, 128], F32, tag="ident")
            make_identity(nc, ident)
            ident_bf = bpool.tile([128, 128], BF16, tag="ident_bf")
            nc.vector.tensor_copy(ident_bf, ident)

            if with_bias:
                bq_sb = bpool.tile([128, HC], F32, tag="bq")
                bk_sb = bpool.tile([128, HC], F32, tag="bk")
                nc.sync.dma_start(out=bq_sb, in_=bq_d.rearrange("(c p) -> p c", p=128))
                nc.sync.dma_start(out=bk_sb, in_=bk_d.rearrange("(c p) -> p c", p=128))
                bvb_sb = bpool.tile([128, H], F32, tag="bvb")
                bv_bcast = bass.AP(tensor=bv_d.tensor, offset=bv_d.offset,
                                   ap=[[0, 128]] + list(bv_d.ap))
                nc.sync.dma_start(out=bvb_sb, in_=bv_bcast)

            hsTs = {0: hsT0, 1: hsT1}
            mgps = {0: mgp0}
            v_sbs = {}

            def alloc_v(b):
                v_sbs[b] = perb.tile([128, SB, H], BF16, tag="v", bufs=2,
                                     name=f"v_sb{b}")

            def v_proj(b, sb_i, half):
                ps = psA.tile([128, S], F32, tag="ps", name="ps_v")
                for ci in range(HC):
                    nc.tensor.matmul(
                        ps, hsTs[b][ci][:, sb_i * 128:(sb_i + 1) * 128],
                        wv_cis[ci][:, half * 512:(half + 1) * 512],
                        start=(ci == 0), stop=(ci == HC - 1))
                dst = v_sbs[b][:, sb_i, half * 512:(half + 1) * 512]
                if with_bias:
                    nc.vector.tensor_add(
                        dst, ps, bvb_sb[:, half * 512:(half + 1) * 512])
                else:
                    nc.vector.tensor_copy(dst, ps)

            def emit_co(b, co):
                hsT_sb = hsTs[b]
                m_sb, gpT_sb = mgps[b]
                v_sb = v_sbs[b]
                ctx_sb = work.tile([128, SB, 2 * DH], F32, tag="ctxco", bufs=3,
                                   name="ctx_co")
                qT_c = work.tile([128, S], F32R, tag="qTc", bufs=3, name="qT_c")
                kT_c = work.tile([128, S], F32R, tag="kTc", bufs=3, name="kT_c")
                for wi, (w_sb, o_sb) in enumerate(
                        ((wq_cos[co], qT_c), (wk_cos[co], kT_c))):
                    ps = psA.tile([128, S], F32, tag="ps", name="ps_proj")
                    for ci in range(HC):
                        nc.tensor.matmul(
                            ps, w_sb[:, ci, :], hsT_sb[ci],
                            start=(ci == 0), stop=(ci == HC - 1))
                    if with_bias:
                        b_sb = bq_sb if wi == 0 else bk_sb
                        nc.scalar.activation(o_sb, ps, AF.Identity,
                                             bias=b_sb[:, co:co + 1])
                    else:
                        nc.scalar.activation(o_sb, ps, AF.Copy)

                for hh in range(2):
                    h = co * 2 + hh
                    hp = hh * 64
                    masked = work.tile([128, SB, S], sdt, tag="masked", bufs=3,
                                       name="masked")
                    ssum = work.tile([128, SB], F32, tag="ssum", bufs=2, name="ssum")
                    ps_c = psC.tile([128, SB, DH], F32, tag="psc", name="ps_c")
                    for r in range(SB):
                        ps_s = psA.tile([128, S], F32, tag="ps", name="ps_s")
                        nc.tensor.matmul(
                            ps_s, qT_c[hp:hp + 64, r * 128:(r + 1) * 128],
                            kT_c[hp:hp + 64, :], start=True, stop=True)
                        nc.vector.tensor_add(masked[:, r, :], ps_s, m_sb[:, r, :])
                        p_sb = work.tile([128, S], BF16, tag="p", bufs=6, name="p_sb")
                        nc.scalar.activation(p_sb, masked[:, r, :], AF.Exp,
                                             accum_out=ssum[:, r:r + 1])
                        ps_t = psT.tile([128, S], BF16, tag="pst", name="ps_t")
                        for c in range(SB):
                            nc.tensor.transpose(
                                ps_t[:, c * 128:(c + 1) * 128],
                                p_sb[:, c * 128:(c + 1) * 128], ident_bf)
                        pgT = work.tile([128, SB, 128], BF16, tag="pgT", bufs=6,
                                        name="pgT")
                        nc.vector.tensor_mul(
                            pgT, ps_t.rearrange("p (c i) -> p c i", c=SB),
                            gpT_sb[:, :, r * 128:(r + 1) * 128])
                        for c in range(SB):
                            nc.tensor.matmul(
                                ps_c[:, r, :], pgT[:, c, :],
                                v_sb[:, c, h * DH:(h + 1) * DH],
                                start=(c == 0), stop=(c == SB - 1))
                    nc.sync.dma_start(
                        out=scores_d[b, h].rearrange("(r p) j -> p r j", p=128),
                        in_=masked)
                    rs = work.tile([128, SB], F32, tag="rs", bufs=2, name="rs")
                    nc.vector.reciprocal(rs, ssum)
                    rs_b = bass.AP(tensor=rs.tensor, offset=rs.offset,
                                   ap=[list(rs.ap[0]), list(rs.ap[1]), [0, DH]])
                    ctx_slice = ctx_sb.rearrange(
                        "p r (g d) -> p r g d", g=2)[:, :, hh, :]
                    if hh == 0:
                        nc.vector.tensor_mul(ctx_slice, ps_c, rs_b)
                    else:
                        nc.vector.tensor_mul(ctx_slice, ps_c, rs_b)
                csl = slice(co * 2 * DH, (co + 1) * 2 * DH)
                nc.sync.dma_start(
                    out=ctx_d[b].rearrange("(r p) o -> p r o", p=128)[:, :, csl],
                    in_=ctx_sb)

            # ---- hand-scheduled emission --------------------------------------
            alloc_v(0)
            for sb_i in range(SB):
                v_proj(0, sb_i, 0)
            for sb_i in range(SB):
                v_proj(0, sb_i, 1)
            for co in range(5):
                emit_co(0, co)
            mgps[1] = load_mgp(1)
            alloc_v(1)
            v_proj(1, 0, 0)
            emit_co(0, 5)
            v_proj(1, 1, 0)
            emit_co(0, 6)
            v_proj(1, 2, 0)
            emit_co(0, 7)
            v_proj(1, 3, 0)
            for co in range(SB):
                v_proj(1, co, 1)
                emit_co(1, co)
            for co in range(SB, HC):
                emit_co(1, co)
    nc.compile()
    return nc


_NC_CACHE = {}


def _get_nc(with_bias=False):
    key = ("nc", with_bias)
    if key not in _NC_CACHE:
        _NC_CACHE[key] = build_nc(with_bias)
    return _NC_CACHE[key]


def prep_inputs(hidden_states, attention_mask, group_prob, Wq, bq, Wk, bk, Wv, bv):
    """Host-side layout prep + sharding. Returns (in_maps, with_bias)."""
    f = np.float32
    hs = np.asarray(hidden_states, dtype=f)
    hsT = np.ascontiguousarray(hs.transpose(0, 2, 1))
    wqT = np.ascontiguousarray(np.asarray(Wq, dtype=f).T / 8.0)
    wkT = np.ascontiguousarray(np.asarray(Wk, dtype=f).T)
    wvT = np.ascontiguousarray(np.asarray(Wv, dtype=f).T)
    bq8 = np.asarray(bq, dtype=f) / 8.0
    bk_ = np.asarray(bk, dtype=f)
    bv_ = np.asarray(bv, dtype=f)
    with_bias = bool(np.abs(bq8).max() or np.abs(bk_).max() or np.abs(bv_).max())
    keep = (np.asarray(attention_mask)[:, 0] != 0) | np.eye(S, dtype=bool)
    madd = np.where(keep, 0, -np.inf).astype(ml_dtypes.bfloat16)
    gpT = np.ascontiguousarray(
        np.asarray(group_prob, dtype=f).transpose(0, 2, 1)).astype(ml_dtypes.bfloat16)
    in_maps = []
    for i in range(NCORES):
        sl = slice(i * NB, (i + 1) * NB)
        m = {
            "hsT": hsT[sl], "wqT": wqT, "wkT": wkT, "wvT": wvT,
            "madd": madd[sl], "gpT": gpT[sl],
        }
        if with_bias:
            m.update({"bq": bq8, "bk": bk_, "bv": bv_})
        in_maps.append(m)
    return in_maps, with_bias


def kernel(hidden_states, attention_mask, group_prob, Wq, bq, Wk, bk, Wv, bv):
    in_maps, with_bias = prep_inputs(hidden_states, attention_mask, group_prob,
                                     Wq, bq, Wk, bk, Wv, bv)
    nc = _get_nc(with_bias)
    res = None
    last_err = None
    for _attempt in range(3):
        try:
            res = run_bass_kernel_spmd(nc, in_maps, core_ids=list(range(NCORES)))
            break
        except Exception as e:  # transient NRT device errors: retry
            last_err = e
    if res is None:
        raise last_err
    ctx = np.concatenate([res.results[i]["ctx"] for i in range(NCORES)], axis=0)
    scores = np.concatenate(
        [np.asarray(res.results[i]["scores"], dtype=np.float32)
         for i in range(NCORES)], axis=0)
    return ctx, scores


# revision 25
# speedup vs baseline: 1.4176x; 1.0178x over previous
"""BertSelfAttention (group_prob-scaled probs, mask|diag masking) on 8 TRN2 cores.

Sharding: data-parallel over batch (16 -> 2 per core). Device math per (b, head):
  v = hs @ Wv^T  first, then per hidden-chunk co: qT/kT chunk projections
  (f32r matmuls) immediately followed by attention for heads 2co, 2co+1:
    scores = qT_h^T @ kT_h   (1/8 folded into Wq on host)
    masked = scores + M      (M in {0, -inf}; exact -inf via DVE add)
    p = exp(masked) [bf16] with accumulated row-sum (ACT)
    pgT = transpose(p) * gpT [bf16]  (PE transpose + fused DVE multiply)
    ctx_h = (pg^T)^T @ v_h / rowsum  (bf16 matmuls, fp32 accumulate)
Emission is hand-scheduled: chunked input DMAs on the ACT HWDGE queue so
compute starts after ~1MB, outputs on the SP queue, and batch 1's v/projection
work interleaved into batch 0's attention tail to avoid pipeline bubbles.
Host side is layout-only: transposes (hs^T, W^T, gp^T), 1/8 scale fold (exact,
power of two), additive mask build, bf16 casts for the probability path.
"""

import os
import sys

import numpy as np

for _p in ("/opt/trn_rl_repo", "/root/.axon_site/_ro/trn_rl_repo"):
    if _p not in sys.path and os.path.isdir(_p):
        sys.path.append(_p)

import ml_dtypes
import concourse.bacc as bacc
import concourse.bass as bass
import concourse.tile as tile
from concourse import mybir
from concourse.bass_utils import run_bass_kernel_spmd
from concourse.masks import make_identity

NB = 2          # batches per core
S = 512         # sequence length
H = 1024        # hidden
NH = 16         # heads
DH = 64         # head dim
NCORES = 8
HC = H // 128   # 8 hidden chunks
SB = S // 128   # 4 seq blocks

F32 = mybir.dt.float32
F32R = mybir.dt.float32r
BF16 = mybir.dt.bfloat16


SCORES_BF16 = False


def build_nc(with_bias=False):
    nc = bacc.Bacc("TRN2", target_bir_lowering=False, debug=False)
    AF = mybir.ActivationFunctionType

    hsT_d = nc.dram_tensor("hsT", [NB, H, S], F32R, kind="ExternalInput").ap()
    wqT_d = nc.dram_tensor("wqT", [H, H], F32R, kind="ExternalInput").ap()
    wkT_d = nc.dram_tensor("wkT", [H, H], F32R, kind="ExternalInput").ap()
    wvT_d = nc.dram_tensor("wvT", [H, H], F32R, kind="ExternalInput").ap()
    if with_bias:
        bq_d = nc.dram_tensor("bq", [H], F32, kind="ExternalInput").ap()
        bk_d = nc.dram_tensor("bk", [H], F32, kind="ExternalInput").ap()
        bv_d = nc.dram_tensor("bv", [H], F32, kind="ExternalInput").ap()
    m_d = nc.dram_tensor("madd", [NB, S, S], BF16, kind="ExternalInput").ap()
    gpT_d = nc.dram_tensor("gpT", [NB, S, S], BF16, kind="ExternalInput").ap()
    sdt = BF16 if SCORES_BF16 else F32
    scores_d = nc.dram_tensor("scores", [NB, NH, S, S], sdt,
                              kind="ExternalOutput").ap()
    ctx_d = nc.dram_tensor("ctx", [NB, S, H], F32, kind="ExternalOutput").ap()

    with tile.TileContext(nc) as tc:
        with (
            tc.tile_pool(name="wpool", bufs=1) as wpool,
            tc.tile_pool(name="bpool", bufs=1) as bpool,
            tc.tile_pool(name="perb", bufs=1) as perb,
            tc.tile_pool(name="work", bufs=1) as work,
            tc.tile_pool(name="psA", bufs=5, space="PSUM") as psA,
            tc.tile_pool(name="psT", bufs=2, space="PSUM") as psT,
            tc.tile_pool(name="psC", bufs=1, space="PSUM") as psC,
        ):
            # ---- DMA plumbing: per-chunk tiles, inputs split across queues -----
            wqT_r = wqT_d.rearrange("(c p) o -> p c o", p=128)
            wkT_r = wkT_d.rearrange("(c p) o -> p c o", p=128)
            wvT_r = wvT_d.rearrange("(c p) o -> p c o", p=128)

            wv_cis, wq_cos, wk_cos = [], [], []
            for ci in range(HC):
                t = wpool.tile([128, H], F32R, tag=f"wv{ci}", name=f"wv_ci{ci}")
                wv_cis.append(t)
            for co in range(HC):
                tq = wpool.tile([128, HC, 128], F32R, tag=f"wq{co}",
                                name=f"wq_co{co}")
                tk = wpool.tile([128, HC, 128], F32R, tag=f"wk{co}",
                                name=f"wk_co{co}")
                wq_cos.append(tq)
                wk_cos.append(tk)

            def load_hsT(b, queue):
                ts = []
                hsT_r = hsT_d[b].rearrange("(c p) s -> p c s", p=128)
                for ci in range(HC):
                    t = perb.tile([128, S], F32R, tag=f"hsT{ci}", bufs=2,
                                  name=f"hsT_b{b}c{ci}")
                    queue.dma_start(out=t, in_=hsT_r[:, ci, :])
                    ts.append(t)
                return ts

            def load_mgp(b):
                mt = perb.tile([128, SB, S], BF16, tag="m", name=f"m_sb{b}")
                nc.scalar.dma_start(
                    out=mt, in_=m_d[b].rearrange("(r p) j -> p r j", p=128))
                gt = perb.tile([128, SB, S], BF16, tag="gpT", name=f"gpT_sb{b}")
                nc.scalar.dma_start(
                    out=gt, in_=gpT_d[b].rearrange("(c p) i -> p c i", p=128))
                return mt, gt

            # scalar queue: wv + hsT0 + m/gp; sync queue: wq/wk + hsT1
            hsT0 = []
            for ci in range(HC):
                nc.scalar.dma_start(out=wv_cis[ci], in_=wvT_r[:, ci, :])
                t = perb.tile([128, S], F32R, tag=f"hsT{ci}", bufs=2,
                              name=f"hsT_b0c{ci}")
                nc.scalar.dma_start(
                    out=t,
                    in_=hsT_d[0].rearrange("(c p) s -> p c s", p=128)[:, ci, :])
                hsT0.append(t)
            mgp0 = load_mgp(0)
            for co in range(HC):
                sl = slice(co * 128, (co + 1) * 128)
                nc.gpsimd.dma_start(out=wq_cos[co], in_=wqT_r[:, :, sl])
                nc.gpsimd.dma_start(out=wk_cos[co], in_=wkT_r[:, :, sl])
            hsT1 = load_hsT(1, nc.gpsimd)

            ident = bpool.tile([128, 128], F32, tag="ident")
            make_identity(nc, ident)
            ident_bf = bpool.tile([128, 128], BF16, tag="ident_bf")
            nc.vector.tensor_copy(ident_bf, ident)

            if with_bias:
                bq_sb = bpool.tile([128, HC], F32, tag="bq")
                bk_sb = bpool.tile([128, HC], F32, tag="bk")
                nc.sync.dma_start(out=bq_sb, in_=bq_d.rearrange("(c p) -> p c", p=128))
                nc.sync.dma_start(out=bk_sb, in_=bk_d.rearrange("(c p) -> p c", p=128))
                bvb_sb = bpool.tile([128, H], F32, tag="bvb")
                bv_bcast = bass.AP(tensor=bv_d.tensor, offset=bv_d.offset,
                                   ap=[[0, 128]] + list(bv_d.ap))
                nc.sync.dma_start(out=bvb_sb, in_=bv_bcast)

            hsTs = {0: hsT0, 1: hsT1}
            mgps = {0: mgp0}
            v_sbs = {}

            def alloc_v(b):
                v_sbs[b] = perb.tile([128, SB, H], BF16, tag="v", bufs=2,
                                     name=f"v_sb{b}")

            def v_proj(b, sb_i, half):
                ps = psA.tile([128, S], F32, tag="ps", name="ps_v")
                for ci in range(HC):
                    nc.tensor.matmul(
                        ps, hsTs[b][ci][:, sb_i * 128:(sb_i + 1) * 128],
                        wv_cis[ci][:, half * 512:(half + 1) * 512],
                        start=(ci == 0), stop=(ci == HC - 1))
                dst = v_sbs[b][:, sb_i, half * 512:(half + 1) * 512]
                if with_bias:
                    nc.vector.tensor_add(
                        dst, ps, bvb_sb[:, half * 512:(half + 1) * 512])
                else:
                    nc.vector.tensor_copy(dst, ps)

            def emit_co(b, co):
                hsT_sb = hsTs[b]
                m_sb, gpT_sb = mgps[b]
                v_sb = v_sbs[b]
                ctx_sb = work.tile([128, SB, 2 * DH], F32, tag="ctxco", bufs=3,
                                   name="ctx_co")
                qT_c = work.tile([128, S], F32R, tag="qTc", bufs=3, name="qT_c")
                kT_c = work.tile([128, S], F32R, tag="kTc", bufs=3, name="kT_c")
                for wi, (w_sb, o_sb) in enumerate(
                        ((wq_cos[co], qT_c), (wk_cos[co], kT_c))):
                    ps = psA.tile([128, S], F32, tag="ps", name="ps_proj")
                    for ci in range(HC):
                        nc.tensor.matmul(
                            ps, w_sb[:, ci, :], hsT_sb[ci],
                            start=(ci == 0), stop=(ci == HC - 1))
                    if with_bias:
                        b_sb = bq_sb if wi == 0 else bk_sb
                        nc.scalar.activation(o_sb, ps, AF.Identity,
                                             bias=b_sb[:, co:co + 1])
                    else:
                        nc.scalar.activation(o_sb, ps, AF.Copy)

                for hh in range(2):
                    h = co * 2 + hh
                    hp = hh * 64
                    masked = work.tile([128, SB, S], sdt, tag="masked", bufs=3,
                                       name="masked")
                    ssum = work.tile([128, SB], F32, tag="ssum", bufs=2, name="ssum")
                    ps_c = psC.tile([128, SB, DH], F32, tag="psc", name="ps_c")
                    for rp in range(SB // 2):
                        r0 = rp * 2
                        ps_t = psT.tile([128, 2, S], BF16, tag="pst", name="ps_t")
                        for rr in range(2):
                            r = r0 + rr
                            ps_s = psA.tile([128, S], F32, tag="ps", name="ps_s")
                            nc.tensor.matmul(
                                ps_s, qT_c[hp:hp + 64, r * 128:(r + 1) * 128],
                                kT_c[hp:hp + 64, :], start=True, stop=True)
                            nc.vector.tensor_add(masked[:, r, :], ps_s,
                                                 m_sb[:, r, :])
                            p_sb = work.tile([128, S], BF16, tag="p", bufs=4,
                                             name="p_sb")
                            nc.scalar.activation(p_sb, masked[:, r, :], AF.Exp,
                                                 accum_out=ssum[:, r:r + 1])
                            for c in range(SB):
                                nc.tensor.transpose(
                                    ps_t[:, rr, c * 128:(c + 1) * 128],
                                    p_sb[:, c * 128:(c + 1) * 128], ident_bf)
                        pgT = work.tile([128, 2, SB, 128], BF16, tag="pgT", bufs=3,
                                        name="pgT")
                        gp2 = bass.AP(
                            tensor=gpT_sb.tensor, offset=gpT_sb.offset + r0 * 128,
                            ap=[list(gpT_sb.ap[0]), [128, 2],
                                list(gpT_sb.ap[1]), [1, 128]])
                        nc.vector.tensor_mul(
                            pgT, ps_t.rearrange("p rr (c i) -> p rr c i", c=SB),
                            gp2)
                        for rr in range(2):
                            for c in range(SB):
                                nc.tensor.matmul(
                                    ps_c[:, r0 + rr, :], pgT[:, rr, c, :],
                                    v_sb[:, c, h * DH:(h + 1) * DH],
                                    start=(c == 0), stop=(c == SB - 1))
                    outq = nc.sync if hh == 0 else nc.scalar
                    outq.dma_start(
                        out=scores_d[b, h].rearrange("(r p) j -> p r j", p=128),
                        in_=masked)
                    rs = work.tile([128, SB], F32, tag="rs", bufs=2, name="rs")
                    nc.vector.reciprocal(rs, ssum)
                    rs_b = bass.AP(tensor=rs.tensor, offset=rs.offset,
                                   ap=[list(rs.ap[0]), list(rs.ap[1]), [0, DH]])
                    ctx_slice = ctx_sb.rearrange(
                        "p r (g d) -> p r g d", g=2)[:, :, hh, :]
                    nc.vector.tensor_mul(ctx_slice, ps_c, rs_b)
                csl = slice(co * 2 * DH, (co + 1) * 2 * DH)
                nc.sync.dma_start(
                    out=ctx_d[b].rearrange("(r p) o -> p r o", p=128)[:, :, csl],
                    in_=ctx_sb)

            # ---- hand-scheduled emission --------------------------------------
            alloc_v(0)
            for sb_i in range(SB):
                v_proj(0, sb_i, 0)
            for sb_i in range(SB):
                v_proj(0, sb_i, 1)
            for co in range(5):
                emit_co(0, co)
            mgps[1] = load_mgp(1)
            alloc_v(1)
            v_proj(1, 0, 0)
            emit_co(0, 5)
            v_proj(1, 1, 0)
            emit_co(0, 6)
            v_proj(1, 2, 0)
            emit_co(0, 7)
            v_proj(1, 3, 0)
            for co in range(SB):
                v_proj(1, co, 1)
                emit_co(1, co)
            for co in range(SB, HC):
                emit_co(1, co)
    nc.compile()
    return nc


_NC_CACHE = {}


def _get_nc(with_bias=False):
    key = ("nc", with_bias)
    if key not in _NC_CACHE:
        _NC_CACHE[key] = build_nc(with_bias)
    return _NC_CACHE[key]


def prep_inputs(hidden_states, attention_mask, group_prob, Wq, bq, Wk, bk, Wv, bv):
    """Host-side layout prep + sharding. Returns (in_maps, with_bias)."""
    f = np.float32
    hs = np.asarray(hidden_states, dtype=f)
    hsT = np.ascontiguousarray(hs.transpose(0, 2, 1))
    wqT = np.ascontiguousarray(np.asarray(Wq, dtype=f).T / 8.0)
    wkT = np.ascontiguousarray(np.asarray(Wk, dtype=f).T)
    wvT = np.ascontiguousarray(np.asarray(Wv, dtype=f).T)
    bq8 = np.asarray(bq, dtype=f) / 8.0
    bk_ = np.asarray(bk, dtype=f)
    bv_ = np.asarray(bv, dtype=f)
    with_bias = bool(np.abs(bq8).max() or np.abs(bk_).max() or np.abs(bv_).max())
    keep = (np.asarray(attention_mask)[:, 0] != 0) | np.eye(S, dtype=bool)
    madd = np.where(keep, 0, -np.inf).astype(ml_dtypes.bfloat16)
    gpT = np.ascontiguousarray(
        np.asarray(group_prob, dtype=f).transpose(0, 2, 1)).astype(ml_dtypes.bfloat16)
    in_maps = []
    for i in range(NCORES):
        sl = slice(i * NB, (i + 1) * NB)
        m = {
            "hsT": hsT[sl], "wqT": wqT, "wkT": wkT, "wvT": wvT,
            "madd": madd[sl], "gpT": gpT[sl],
        }
        if with_bias:
            m.update({"bq": bq8, "bk": bk_, "bv": bv_})
        in_maps.append(m)
    return in_maps, with_bias


def kernel(hidden_states, attention_mask, group_prob, Wq, bq, Wk, bk, Wv, bv):
    in_maps, with_bias = prep_inputs(hidden_states, attention_mask, group_prob,
                                     Wq, bq, Wk, bk, Wv, bv)
    nc = _get_nc(with_bias)
    res = None
    last_err = None
    for _attempt in range(3):
        try:
            res = run_bass_kernel_spmd(nc, in_maps, core_ids=list(range(NCORES)))
            break
        except Exception as e:  # transient NRT device errors: retry
            last_err = e
    if res is None:
        raise last_err
    ctx = np.concatenate([res.results[i]["ctx"] for i in range(NCORES)], axis=0)
    scores = np.concatenate(
        [np.asarray(res.results[i]["scores"], dtype=np.float32)
         for i in range(NCORES)], axis=0)
    return ctx, scores


# revision 31
# speedup vs baseline: 1.4242x; 1.0047x over previous
"""BertSelfAttention (group_prob-scaled probs, mask|diag masking) on 8 TRN2 cores.

Sharding: data-parallel over batch (16 -> 2 per core). Device math per (b, head):
  v = hs @ Wv^T  first, then per hidden-chunk co: qT/kT chunk projections
  (f32r matmuls) immediately followed by attention for heads 2co, 2co+1:
    scores = qT_h^T @ kT_h   (1/8 folded into Wq on host)
    masked = scores + M      (M in {0, -inf}; exact -inf via DVE add)
    p = exp(masked) [bf16] with accumulated row-sum (ACT)
    pgT = transpose(p) * gpT [bf16]  (PE transpose + fused DVE multiply)
    ctx_h = (pg^T)^T @ v_h / rowsum  (bf16 matmuls, fp32 accumulate)
Emission is hand-scheduled: chunked input DMAs on the ACT HWDGE queue so
compute starts after ~1MB, outputs on the SP queue, and batch 1's v/projection
work interleaved into batch 0's attention tail to avoid pipeline bubbles.
Host side is layout-only: transposes (hs^T, W^T, gp^T), 1/8 scale fold (exact,
power of two), additive mask build, bf16 casts for the probability path.
"""

import os
import sys

import numpy as np

for _p in ("/opt/trn_rl_repo", "/root/.axon_site/_ro/trn_rl_repo"):
    if _p not in sys.path and os.path.isdir(_p):
        sys.path.append(_p)

import ml_dtypes
import concourse.bacc as bacc
import concourse.bass as bass
import concourse.tile as tile
from concourse import mybir
from concourse.bass_utils import run_bass_kernel_spmd
from concourse.masks import make_identity

NB = 2          # batches per core
S = 512         # sequence length
H = 1024        # hidden
NH = 16         # heads
DH = 64         # head dim
NCORES = 8
HC = H // 128   # 8 hidden chunks
SB = S // 128   # 4 seq blocks

F32 = mybir.dt.float32
F32R = mybir.dt.float32r
BF16 = mybir.dt.bfloat16


SCORES_BF16 = False


def build_nc(with_bias=False):
    nc = bacc.Bacc("TRN2", target_bir_lowering=False, debug=False)
    AF = mybir.ActivationFunctionType

    hsT_d = nc.dram_tensor("hsT", [NB, H, S], F32R, kind="ExternalInput").ap()
    wqT_d = nc.dram_tensor("wqT", [H, H], F32R, kind="ExternalInput").ap()
    wkT_d = nc.dram_tensor("wkT", [H, H], F32R, kind="ExternalInput").ap()
    wvT_d = nc.dram_tensor("wvT", [H, H], F32R, kind="ExternalInput").ap()
    if with_bias:
        bq_d = nc.dram_tensor("bq", [H], F32, kind="ExternalInput").ap()
        bk_d = nc.dram_tensor("bk", [H], F32, kind="ExternalInput").ap()
        bv_d = nc.dram_tensor("bv", [H], F32, kind="ExternalInput").ap()
    m_d = nc.dram_tensor("madd", [NB, S, S], BF16, kind="ExternalInput").ap()
    gpT_d = nc.dram_tensor("gpT", [NB, S, S], BF16, kind="ExternalInput").ap()
    sdt = BF16 if SCORES_BF16 else F32
    scores_d = nc.dram_tensor("scores", [NB, NH, S, S], sdt,
                              kind="ExternalOutput").ap()
    ctx_d = nc.dram_tensor("ctx", [NB, S, H], F32, kind="ExternalOutput").ap()

    with tile.TileContext(nc) as tc:
        with (
            tc.tile_pool(name="wpool", bufs=1) as wpool,
            tc.tile_pool(name="bpool", bufs=1) as bpool,
            tc.tile_pool(name="perb", bufs=1) as perb,
            tc.tile_pool(name="work", bufs=1) as work,
            tc.tile_pool(name="psA", bufs=5, space="PSUM") as psA,
            tc.tile_pool(name="psT", bufs=2, space="PSUM") as psT,
            tc.tile_pool(name="psC", bufs=1, space="PSUM") as psC,
        ):
            # ---- DMA plumbing: per-chunk tiles, inputs split across queues -----
            wqT_r = wqT_d.rearrange("(c p) o -> p c o", p=128)
            wkT_r = wkT_d.rearrange("(c p) o -> p c o", p=128)
            wvT_r = wvT_d.rearrange("(c p) o -> p c o", p=128)

            wv_cis, wq_cos, wk_cos = [], [], []
            for ci in range(HC):
                t = wpool.tile([128, H], F32R, tag=f"wv{ci}", name=f"wv_ci{ci}")
                wv_cis.append(t)
            for co in range(HC):
                tq = wpool.tile([128, HC, 128], F32R, tag=f"wq{co}",
                                name=f"wq_co{co}")
                tk = wpool.tile([128, HC, 128], F32R, tag=f"wk{co}",
                                name=f"wk_co{co}")
                wq_cos.append(tq)
                wk_cos.append(tk)

            def load_hsT(b, queue):
                ts = []
                hsT_r = hsT_d[b].rearrange("(c p) s -> p c s", p=128)
                for ci in range(HC):
                    t = perb.tile([128, S], F32R, tag=f"hsT{ci}", bufs=2,
                                  name=f"hsT_b{b}c{ci}")
                    queue.dma_start(out=t, in_=hsT_r[:, ci, :])
                    ts.append(t)
                return ts

            def load_mgp(b):
                mt = perb.tile([128, SB, S], BF16, tag="m", name=f"m_sb{b}")
                nc.scalar.dma_start(
                    out=mt, in_=m_d[b].rearrange("(r p) j -> p r j", p=128))
                gt = perb.tile([128, SB, S], BF16, tag="gpT", name=f"gpT_sb{b}")
                nc.scalar.dma_start(
                    out=gt, in_=gpT_d[b].rearrange("(c p) i -> p c i", p=128))
                return mt, gt

            # scalar queue: wv + hsT0 + m/gp; sync queue: wq/wk + hsT1
            hsT0 = []
            for ci in range(HC):
                nc.sync.dma_start(out=wv_cis[ci], in_=wvT_r[:, ci, :])
                t = perb.tile([128, S], F32R, tag=f"hsT{ci}", bufs=2,
                              name=f"hsT_b0c{ci}")
                nc.scalar.dma_start(
                    out=t,
                    in_=hsT_d[0].rearrange("(c p) s -> p c s", p=128)[:, ci, :])
                hsT0.append(t)
            mgp0 = load_mgp(0)
            for co in range(HC):
                sl = slice(co * 128, (co + 1) * 128)
                nc.gpsimd.dma_start(out=wq_cos[co], in_=wqT_r[:, :, sl])
                nc.gpsimd.dma_start(out=wk_cos[co], in_=wkT_r[:, :, sl])
            hsT1 = load_hsT(1, nc.gpsimd)

            ident = bpool.tile([128, 128], F32, tag="ident")
            make_identity(nc, ident)
            ident_bf = bpool.tile([128, 128], BF16, tag="ident_bf")
            nc.vector.tensor_copy(ident_bf, ident)

            if with_bias:
                bq_sb = bpool.tile([128, HC], F32, tag="bq")
                bk_sb = bpool.tile([128, HC], F32, tag="bk")
                nc.sync.dma_start(out=bq_sb, in_=bq_d.rearrange("(c p) -> p c", p=128))
                nc.sync.dma_start(out=bk_sb, in_=bk_d.rearrange("(c p) -> p c", p=128))
                bvb_sb = bpool.tile([128, H], F32, tag="bvb")
                bv_bcast = bass.AP(tensor=bv_d.tensor, offset=bv_d.offset,
                                   ap=[[0, 128]] + list(bv_d.ap))
                nc.sync.dma_start(out=bvb_sb, in_=bv_bcast)

            hsTs = {0: hsT0, 1: hsT1}
            mgps = {0: mgp0}
            v_sbs = {}

            def alloc_v(b):
                v_sbs[b] = perb.tile([128, SB, H], BF16, tag="v", bufs=2,
                                     name=f"v_sb{b}")

            def v_proj(b, sb_i, half):
                ps = psA.tile([128, S], F32, tag="ps", name="ps_v")
                for ci in range(HC):
                    nc.tensor.matmul(
                        ps, hsTs[b][ci][:, sb_i * 128:(sb_i + 1) * 128],
                        wv_cis[ci][:, half * 512:(half + 1) * 512],
                        start=(ci == 0), stop=(ci == HC - 1))
                dst = v_sbs[b][:, sb_i, half * 512:(half + 1) * 512]
                if with_bias:
                    nc.vector.tensor_add(
                        dst, ps, bvb_sb[:, half * 512:(half + 1) * 512])
                else:
                    nc.vector.tensor_copy(dst, ps)

            def emit_co(b, co):
                hsT_sb = hsTs[b]
                m_sb, gpT_sb = mgps[b]
                v_sb = v_sbs[b]
                ctx_sb = work.tile([128, SB, 2 * DH], F32, tag="ctxco", bufs=3,
                                   name="ctx_co")
                qT_c = work.tile([128, S], F32R, tag="qTc", bufs=3, name="qT_c")
                kT_c = work.tile([128, S], F32R, tag="kTc", bufs=3, name="kT_c")
                for wi, (w_sb, o_sb) in enumerate(
                        ((wq_cos[co], qT_c), (wk_cos[co], kT_c))):
                    ps = psA.tile([128, S], F32, tag="ps", name="ps_proj")
                    for ci in range(HC):
                        nc.tensor.matmul(
                            ps, w_sb[:, ci, :], hsT_sb[ci],
                            start=(ci == 0), stop=(ci == HC - 1))
                    if with_bias:
                        b_sb = bq_sb if wi == 0 else bk_sb
                        nc.scalar.activation(o_sb, ps, AF.Identity,
                                             bias=b_sb[:, co:co + 1])
                    else:
                        nc.scalar.activation(o_sb, ps, AF.Copy)

                for hh in range(2):
                    h = co * 2 + hh
                    hp = hh * 64
                    masked = work.tile([128, SB, S], sdt, tag="masked", bufs=3,
                                       name="masked")
                    ssum = work.tile([128, SB], F32, tag="ssum", bufs=2, name="ssum")
                    ps_c = psC.tile([128, SB, DH], F32, tag="psc", name="ps_c")
                    for rp in range(SB // 2):
                        r0 = rp * 2
                        ps_t = psT.tile([128, 2, S], BF16, tag="pst", name="ps_t")
                        for rr in range(2):
                            r = r0 + rr
                            ps_s = psA.tile([128, S], F32, tag="ps", name="ps_s")
                            nc.tensor.matmul(
                                ps_s, qT_c[hp:hp + 64, r * 128:(r + 1) * 128],
                                kT_c[hp:hp + 64, :], start=True, stop=True)
                            nc.vector.tensor_add(masked[:, r, :], ps_s,
                                                 m_sb[:, r, :])
                            p_sb = work.tile([128, S], BF16, tag="p", bufs=4,
                                             name="p_sb")
                            nc.scalar.activation(p_sb, masked[:, r, :], AF.Exp,
                                                 accum_out=ssum[:, r:r + 1])
                            for c in range(SB):
                                nc.tensor.transpose(
                                    ps_t[:, rr, c * 128:(c + 1) * 128],
                                    p_sb[:, c * 128:(c + 1) * 128], ident_bf)
                        pgT = work.tile([128, 2, SB, 128], BF16, tag="pgT", bufs=3,
                                        name="pgT")
                        gp2 = bass.AP(
                            tensor=gpT_sb.tensor, offset=gpT_sb.offset + r0 * 128,
                            ap=[list(gpT_sb.ap[0]), [128, 2],
                                list(gpT_sb.ap[1]), [1, 128]])
                        nc.vector.tensor_mul(
                            pgT, ps_t.rearrange("p rr (c i) -> p rr c i", c=SB),
                            gp2)
                        for rr in range(2):
                            for c in range(SB):
                                nc.tensor.matmul(
                                    ps_c[:, r0 + rr, :], pgT[:, rr, c, :],
                                    v_sb[:, c, h * DH:(h + 1) * DH],
                                    start=(c == 0), stop=(c == SB - 1))
                    outq = nc.sync if hh == 0 else nc.scalar
                    outq.dma_start(
                        out=scores_d[b, h].rearrange("(r p) j -> p r j", p=128),
                        in_=masked)
                    rs = work.tile([128, SB], F32, tag="rs", bufs=2, name="rs")
                    nc.vector.reciprocal(rs, ssum)
                    rs_b = bass.AP(tensor=rs.tensor, offset=rs.offset,
                                   ap=[list(rs.ap[0]), list(rs.ap[1]), [0, DH]])
                    ctx_slice = ctx_sb.rearrange(
                        "p r (g d) -> p r g d", g=2)[:, :, hh, :]
                    nc.vector.tensor_mul(ctx_slice, ps_c, rs_b)
                csl = slice(co * 2 * DH, (co + 1) * 2 * DH)
                nc.sync.dma_start(
                    out=ctx_d[b].rearrange("(r p) o -> p r o", p=128)[:, :, csl],
                    in_=ctx_sb)

            # ---- hand-scheduled emission --------------------------------------
            alloc_v(0)
            for sb_i in range(SB):
                v_proj(0, sb_i, 0)
            for sb_i in range(SB):
                v_proj(0, sb_i, 1)
            for co in range(5):
                emit_co(0, co)
            mgps[1] = load_mgp(1)
            alloc_v(1)
            v_proj(1, 0, 0)
            emit_co(0, 5)
            v_proj(1, 1, 0)
            emit_co(0, 6)
            v_proj(1, 2, 0)
            emit_co(0, 7)
            v_proj(1, 3, 0)
            for co in range(SB):
                v_proj(1, co, 1)
                emit_co(1, co)
            for co in range(SB, HC):
                emit_co(1, co)
    nc.compile()
    return nc


_NC_CACHE = {}


def _get_nc(with_bias=False):
    key = ("nc", with_bias)
    if key not in _NC_CACHE:
        _NC_CACHE[key] = build_nc(with_bias)
    return _NC_CACHE[key]


def prep_inputs(hidden_states, attention_mask, group_prob, Wq, bq, Wk, bk, Wv, bv):
    """Host-side layout prep + sharding. Returns (in_maps, with_bias)."""
    f = np.float32
    hs = np.asarray(hidden_states, dtype=f)
    hsT = np.ascontiguousarray(hs.transpose(0, 2, 1))
    wqT = np.ascontiguousarray(np.asarray(Wq, dtype=f).T / 8.0)
    wkT = np.ascontiguousarray(np.asarray(Wk, dtype=f).T)
    wvT = np.ascontiguousarray(np.asarray(Wv, dtype=f).T)
    bq8 = np.asarray(bq, dtype=f) / 8.0
    bk_ = np.asarray(bk, dtype=f)
    bv_ = np.asarray(bv, dtype=f)
    with_bias = bool(np.abs(bq8).max() or np.abs(bk_).max() or np.abs(bv_).max())
    keep = (np.asarray(attention_mask)[:, 0] != 0) | np.eye(S, dtype=bool)
    madd = np.where(keep, 0, -np.inf).astype(ml_dtypes.bfloat16)
    gpT = np.ascontiguousarray(
        np.asarray(group_prob, dtype=f).transpose(0, 2, 1)).astype(ml_dtypes.bfloat16)
    in_maps = []
    for i in range(NCORES):
        sl = slice(i * NB, (i + 1) * NB)
        m = {
            "hsT": hsT[sl], "wqT": wqT, "wkT": wkT, "wvT": wvT,
            "madd": madd[sl], "gpT": gpT[sl],
        }
        if with_bias:
            m.update({"bq": bq8, "bk": bk_, "bv": bv_})
        in_maps.append(m)
    return in_maps, with_bias


def kernel(hidden_states, attention_mask, group_prob, Wq, bq, Wk, bk, Wv, bv):
    in_maps, with_bias = prep_inputs(hidden_states, attention_mask, group_prob,
                                     Wq, bq, Wk, bk, Wv, bv)
    nc = _get_nc(with_bias)
    res = None
    last_err = None
    for _attempt in range(3):
        try:
            res = run_bass_kernel_spmd(nc, in_maps, core_ids=list(range(NCORES)))
            break
        except Exception as e:  # transient NRT device errors: retry
            last_err = e
    if res is None:
        raise last_err
    ctx = np.concatenate([res.results[i]["ctx"] for i in range(NCORES)], axis=0)
    scores = np.concatenate(
        [np.asarray(res.results[i]["scores"], dtype=np.float32)
         for i in range(NCORES)], axis=0)
    return ctx, scores
